# revision 5
# baseline (speedup 1.0000x reference)
"""Cumulative max along axis 2 (W) of [8, 512, 512, 64] f32, on 8 TRN2 NeuronCores.

Sharding: (batch-pair, channel-half) -> each core owns a host-contiguous
[2, 512, 512, 32] slab. 32 channels puts the per-channel W stride in SBUF at
128 B, where the DVE TensorTensorScan runs at its full 2 cyc/elem rate (the
256 B stride of a full-64-channel tile costs ~30% extra). Per core, tiles are
[128 h-partitions, 512 w, 32 c] (64 KB contiguous DRAM run per partition), and
each channel is one full-width hardware scan — no inter-tile carry.
"""
import numpy as np

import concourse.bass as bass
from concourse import bacc, mybir, tile
from concourse.bass_utils import run_bass_kernel_spmd

B, H, W, C = 8, 512, 512, 64
P = 128            # SBUF partitions per h-group
BPC, CPC = 2, 32   # batches / channels per core
N_CORES = 8
NEG = -3.4028234663852886e38  # max identity; -inf doesn't survive BIR JSON

_NC_CACHE = {}


def build_nc(debug=False):
    n_hg = H // P
    nc = bacc.Bacc("TRN2", target_bir_lowering=False, debug=debug)
    x = nc.dram_tensor("x", [BPC, H, W, CPC], mybir.dt.float32, kind="ExternalInput")
    out = nc.dram_tensor("out", [BPC, H, W, CPC], mybir.dt.float32, kind="ExternalOutput")
    with tile.TileContext(nc) as tc:
        with tc.tile_pool(name="data", bufs=3) as pool:
            for b in range(BPC):
                for hg in range(n_hg):
                    t = pool.tile([P, W, CPC], mybir.dt.float32, name="t", tag="data")
                    nc.sync.dma_start(out=t[:], in_=x[b, hg*P:(hg+1)*P, :, :])
                    for c in range(CPC):
                        nc.vector.tensor_tensor_scan(
                            out=t[:, :, c], data0=t[:, :, c], data1=t[:, :, c],
                            initial=NEG,
                            op0=mybir.AluOpType.max, op1=mybir.AluOpType.max,
                        )
                    nc.scalar.dma_start(out=out[b, hg*P:(hg+1)*P, :, :], in_=t[:])
    nc.compile()
    return nc


def get_nc():
    if "nc" not in _NC_CACHE:
        _NC_CACHE["nc"] = build_nc()
    return _NC_CACHE["nc"]


def _shard(x_full):
    # core k -> batches [2*(k%4), 2*(k%4)+2), channels [32*(k//4), 32*(k//4)+32)
    maps = []
    for k in range(N_CORES):
        b0, c0 = 2 * (k % 4), CPC * (k // 4)
        maps.append({"x": np.ascontiguousarray(x_full[b0:b0+2, :, :, c0:c0+CPC])})
    return maps


def run_spmd(x_full, trace=False, **kwargs):
    nc = get_nc()
    maps = _shard(x_full)
    last_err = None
    for _attempt in range(3):
        try:
            res = run_bass_kernel_spmd(nc, maps, list(range(N_CORES)),
                                       trace=trace, **kwargs)
            break
        except Exception as e:  # transient NRT device errors recover on retry
            last_err = e
    else:
        raise last_err
    out = np.empty((B, H, W, C), dtype=np.float32)
    for k in range(N_CORES):
        b0, c0 = 2 * (k % 4), CPC * (k // 4)
        out[b0:b0+2, :, :, c0:c0+CPC] = res.results[k]["out"]
    return out, res


def kernel(**inputs):
    x = np.asarray(inputs["inputs"], dtype=np.float32)
    assert x.shape == (B, H, W, C), x.shape
    out, _ = run_spmd(x)
    return out
